# revision 2
# baseline (speedup 1.0000x reference)
"""Trainium2 Bass kernel for dense_cnn problem.

Math (per batch element n, C=128 channels, H=W=56, G=8):
  t1 = conv_h(x, w1)          5-tap conv over H with full channel mixing
  t3 = dwconv_h(t1, w3)       3-tap depthwise conv over H
  t4[g] = sum_{c,k} x[c, h, w+2k-2] * w4[c,k,g]   (3 width taps, dil 2)
  out[c] = t3[c] * t4[c % 8]

Device strategy (data-parallel, 4 batch elems per core across 8 cores):
  - PE does the dense work: t1 as a 5-tap conv (clipped shifted matmuls)
    and t4 broadcast to 128 channels (3 taps) -> 8 column passes per
    chunk.  That is the engine floor (~42us @ 2.4GHz); everything else
    is arranged to stay off the PE critical path.
  - Depthwise 3-tap via one scaled ACT copy + two DVE STTs in bf16:
      t1s = w3[c,1] * t1                      (ACT, PSUM->SBUF, bf16)
      q   = (w30/w31) * t1s[h-1] + t1s[h]     (DVE STT, bf16 2x mode)
      t3  = (w32/w31) * t1s[h+1] + q          (DVE STT, bf16 2x mode)
      out = t3 * t4(psum)                     (DVE TT, fp16 out)
    t1s has zero pad rows, so no border special cases.  STTs are
    batched over 2-chunk granules to amortize the fixed op overhead.
    GpSimd only zeroes the pad rows; it is off the critical path
    (its fp32 adds measured 1.49us/chunk - as slow as the PE).
  - t3 for chunk c needs t1s row h0+8 from chunk c+1's copy, so the
    STT2/mul/store pipeline runs one granule behind the PE.
  - No PE warm-up dummies: HAM ungates the 2.4 GHz clock after ~3.4us
    of activity regardless of what runs, so real matmuls burn the cold
    window.  Input DMAs are issued finest-first so chunk 0's data lands
    before the engine instruction queues finish loading (~5us).
  - Matmuls in bf16 (fp32 matmul lowers to a LOW_HIGH pair at <half
    throughput); accumulation stays fp32 in PSUM.
  - Output written fp16 (half the DMA bytes), widened on host.
"""

import sys

sys.path.insert(0, "/opt/trn_rl_repo")

import ml_dtypes
import numpy as np

import concourse.bacc as bacc
import concourse.bass as bass
import concourse.mybir as mybir
import concourse.tile as tile
from concourse import bass_utils

N, C, H, W, G = 32, 128, 56, 56, 8
NCORES = 8
NPC = N // NCORES  # batch elems per core
CH = 8             # H rows per chunk
NCHUNK = H // CH

F32 = mybir.dt.float32
F16 = mybir.dt.float16
BF16 = mybir.dt.bfloat16

TRACE = False
TRACE_DIR = None
LAST_EXEC_NS = None
LAST_RESULTS = None

_COMPILED = None


def _enable_trace_hook():
    """The agent image's ``antenv`` lacks ``axon_hooks``, so the boot-time
    NTFF hook registration silently degraded. Recreate the module and
    register the same ctypes-based hook; also skip the bucket upload."""
    import sys as _sys
    import types

    if "antenv.axon_hooks" not in _sys.modules:
        mod = types.ModuleType("antenv.axon_hooks")
        mod._hook = None

        def set_axon_ntff_profile_hook(h):
            mod._hook = h

        def get_axon_ntff_profile_hook():
            return mod._hook

        mod.set_axon_ntff_profile_hook = set_axon_ntff_profile_hook
        mod.get_axon_ntff_profile_hook = get_axon_ntff_profile_hook
        _sys.modules["antenv.axon_hooks"] = mod
        import antenv

        antenv.axon_hooks = mod

    from antenv.axon_hooks import get_axon_ntff_profile_hook as _get

    if _get() is None:
        from trn_agent_boot.trn_boot import _ntff_profile_via_ctypes

        hook = _ntff_profile_via_ctypes("/opt/axon/libaxon_pjrt.so")
        if hook is not None:
            _sys.modules["antenv.axon_hooks"].set_axon_ntff_profile_hook(hook)

    bass_utils.upload_artifacts = lambda tmpdir: f"local:{tmpdir}"


def _t1_matmuls(c, pa, xc, wc_t):
    """5-tap H-conv for chunk c with row clipping at the H borders.
    Output row o of the chunk reads x row 8c+o+e-2 for tap e."""
    h0 = c * CH
    mms = []
    # e=2 covers the full chunk for every c -> emitted first (start=True)
    for e in (2, 0, 1, 3, 4):
        o_lo = max(0, 2 - e - h0)
        o_hi = min(CH, H + 2 - e - h0)
        if o_lo >= o_hi:
            continue
        r0 = h0 + o_lo + e - 2
        r1 = h0 + o_hi + e - 2
        mms.append((wc_t[:, e, :], xc[:, r0:r1, :], pa[:, o_lo:o_hi, :]))
    return mms


def _t4_matmuls(c, pb, xc, w4_t):
    """t4 chunk: 3 width taps at offsets -2/0/+2, col-clipped at borders."""
    h0 = c * CH
    rows = xc[:, h0 : h0 + CH, :]
    return [
        (w4_t[:, 1, :], rows, pb[:]),                               # delta = 0
        (w4_t[:, 0, :], xc[:, h0 : h0 + CH, 0 : W - 2], pb[:, :, 2:W]),   # -2
        (w4_t[:, 2, :], xc[:, h0 : h0 + CH, 2:W], pb[:, :, 0 : W - 2]),   # +2
    ]


def _build():
    nc = bacc.Bacc(
        "TRN2",
        target_bir_lowering=False,
        debug=False,
        enable_asserts=False,
        num_devices=NCORES,
    )

    x_d = nc.dram_tensor("x_s", (NPC, C, H, W), BF16, kind="ExternalInput").ap()
    wc_d = nc.dram_tensor("wc5", (C, 5, C), BF16, kind="ExternalInput").ap()
    w4_d = nc.dram_tensor("w4b", (C, 3, C), BF16, kind="ExternalInput").ap()
    sc_d = nc.dram_tensor("scal", (C, 3), F32, kind="ExternalInput").ap()
    out_d = nc.dram_tensor("out", (NPC, C, H, W), F16, kind="ExternalOutput").ap()

    mult = mybir.AluOpType.mult
    add = mybir.AluOpType.add
    COPY = mybir.ActivationFunctionType.Copy

    # 2-chunk back-end granules (last one is a 1-chunk remainder)
    GRAN = [(0, 2), (2, 2), (4, 2), (6, 1)]

    with tile.TileContext(nc) as tc:
        with (
            tc.tile_pool(name="wpool", bufs=1) as wpool,
            tc.tile_pool(name="xpool", bufs=1) as xpool,
            tc.tile_pool(name="t1pool", bufs=2) as t1pool,
            tc.tile_pool(name="qpool", bufs=3) as qpool,
            tc.tile_pool(name="t3pool", bufs=3) as t3pool,
            tc.tile_pool(name="opool", bufs=3) as opool,
            tc.tile_pool(name="psA", bufs=3, space="PSUM") as papool,
            tc.tile_pool(name="psB", bufs=5, space="PSUM") as pbpool,
        ):
            wc_t = wpool.tile([C, 5, C], BF16)
            w4_t = wpool.tile([C, 3, C], BF16)
            sc_t = wpool.tile([C, 3], F32)
            warm = wpool.tile([1, 1], F32)

            xcs = []
            for n in range(NPC):
                xc = xpool.tile([C, H, W], BF16, name=f"xc{n}")
                xcs.append(xc)

            # DMA order: weights first (needed by the very first LDWEIGHTS),
            # then batch 0 in fine row slices so chunk 0 can start ASAP.
            nc.sync.dma_start(wc_t[:], wc_d[:])
            nc.sync.dma_start(xcs[0][:, 0:10, :], x_d[0, :, 0:10, :])
            nc.sync.dma_start(sc_t[:], sc_d[:])
            nc.sync.dma_start(w4_t[:], w4_d[:])
            nc.sync.dma_start(xcs[0][:, 10:28, :], x_d[0, :, 10:28, :])
            nc.sync.dma_start(xcs[0][:, 28:56, :], x_d[0, :, 28:56, :])
            for n in range(1, NPC):
                nc.sync.dma_start(xcs[n][:, 0:28, :], x_d[n, :, 0:28, :])
                nc.sync.dma_start(xcs[n][:, 28:56, :], x_d[n, :, 28:56, :])

            # Trip the one-time ACT_TABLE_LOAD (~1.3us) before the first
            # real copy needs it.
            nc.scalar.activation(warm[:], sc_t[0:1, 0:1], COPY)

            w31 = sc_t[:, 0:1]
            s0 = sc_t[:, 1:2]
            s2 = sc_t[:, 2:3]

            for n in range(NPC):
                xc = xcs[n]
                last_n = n == NPC - 1

                # t1s rows: 0 = zero pad (h=-1), 1..56 = h, 57 = zero pad
                t1s = t1pool.tile([C, H + 2, W], BF16, name="t1s")
                nc.gpsimd.memset(t1s[:, 0:1, :], 0.0)
                nc.gpsimd.memset(t1s[:, H + 1 : H + 2, :], 0.0)

                pbs = [None] * NCHUNK
                qs = {}
                t3s = {}

                def front(c):
                    """PE matmuls + scaled ACT copy into the t1s halo."""
                    h0 = c * CH
                    pa = papool.tile([C, CH, W], F32, name="pa")
                    mms = _t1_matmuls(c, pa, xc, wc_t)
                    for i, (lhsT, rhs, outap) in enumerate(mms):
                        nc.tensor.matmul(
                            outap, lhsT=lhsT, rhs=rhs,
                            start=(i == 0), stop=(i == len(mms) - 1),
                        )
                    pb = pbpool.tile([C, CH, W], F32, name="pb")
                    for i, (lhsT, rhs, outap) in enumerate(_t4_matmuls(c, pb, xc, w4_t)):
                        nc.tensor.matmul(
                            outap, lhsT=lhsT, rhs=rhs,
                            start=(i == 0), stop=(i == 2),
                        )
                    pbs[c] = pb
                    # t1s[1+h0 : 1+h0+CH] = w3_1 * t1   (per-partition scale)
                    nc.scalar.activation(
                        t1s[:, 1 + h0 : 1 + h0 + CH, :], pa[:], COPY, scale=w31
                    )

                def stt1(g):
                    """q = (w30/w31)*t1s[h-1] + t1s[h] over granule g (bf16)."""
                    c0, ln = GRAN[g]
                    h0 = c0 * CH
                    rows = ln * CH
                    q = qpool.tile([C, 2 * CH, W], BF16, name="q")
                    nc.vector.scalar_tensor_tensor(
                        q[:, 0:rows, :],
                        t1s[:, h0 : h0 + rows, :],
                        s0,
                        t1s[:, 1 + h0 : 1 + h0 + rows, :],
                        op0=mult, op1=add,
                    )
                    qs[g] = q

                def stt2(g):
                    """t3 = (w32/w31)*t1s[h+1] + q over granule g (bf16).
                    Needs t1s row h0+rows+1, i.e. the next granule's first
                    ACT copy (zero pad row for the last granule)."""
                    c0, ln = GRAN[g]
                    h0 = c0 * CH
                    rows = ln * CH
                    t3 = t3pool.tile([C, 2 * CH, W], BF16, name="t3")
                    nc.vector.scalar_tensor_tensor(
                        t3[:, 0:rows, :],
                        t1s[:, 2 + h0 : 2 + h0 + rows, :],
                        s2,
                        qs[g][:, 0:rows, :],
                        op0=mult, op1=add,
                    )
                    t3s[g] = t3

                def muls(g):
                    """Final multiply per chunk (pb is a per-chunk PSUM tile),
                    then store the granule (last batch elem: per chunk)."""
                    c0, ln = GRAN[g]
                    ot = opool.tile([C, 2 * CH, W], F16, name="ot")
                    for j in range(ln):
                        c = c0 + j
                        sl = ot[:, j * CH : (j + 1) * CH, :]
                        nc.vector.tensor_mul(
                            sl, t3s[g][:, j * CH : (j + 1) * CH, :], pbs[c][:]
                        )
                        if last_n:
                            nc.sync.dma_start(
                                out_d[n, :, c * CH : (c + 1) * CH, :], sl
                            )
                    if not last_n:
                        nc.sync.dma_start(
                            out_d[n, :, c0 * CH : (c0 + ln) * CH, :],
                            ot[:, 0 : ln * CH, :],
                        )

                front(0)
                front(1)
                stt1(0)
                front(2)
                stt2(0)
                muls(0)
                front(3)
                stt1(1)
                front(4)
                stt2(1)
                muls(1)
                front(5)
                stt1(2)
                front(6)
                stt1(3)
                stt2(2)
                muls(2)
                stt2(3)
                muls(3)

    nc.compile()
    return nc


def _get_compiled():
    global _COMPILED
    if _COMPILED is None:
        _COMPILED = _build()
    return _COMPILED


def _prep_weights(w1, w3, w4):
    bf = ml_dtypes.bfloat16
    w1c = np.asarray(w1, dtype=np.float32)[:, :, :, 0]  # (co, ci, 5)
    wc5 = np.ascontiguousarray(np.transpose(w1c, (1, 2, 0))).astype(bf)  # (ci,e,co)
    w4c = np.asarray(w4, dtype=np.float32)[:, :, 0, :]  # (ci, k, g)
    w4b = np.ascontiguousarray(np.tile(w4c, (1, 1, C // G))).astype(bf)
    w3c = np.asarray(w3, dtype=np.float32)[:, 0, :, 0]  # (co, 3)
    w31 = w3c[:, 1].copy()
    w31[np.abs(w31) < 1e-12] = 1e-12
    scal = np.stack([w31, w3c[:, 0] / w31, w3c[:, 2] / w31], axis=1)
    return wc5, w4b, np.ascontiguousarray(scal, dtype=np.float32)


def kernel(x, w1, w3, w4):
    global LAST_EXEC_NS, LAST_RESULTS
    nc = _get_compiled()
    xb = np.ascontiguousarray(np.asarray(x, dtype=np.float32)).astype(ml_dtypes.bfloat16)
    wc5, w4b, scal = _prep_weights(w1, w3, w4)

    in_maps = [
        {
            "x_s": np.ascontiguousarray(xb[i * NPC : (i + 1) * NPC]),
            "wc5": wc5,
            "w4b": w4b,
            "scal": scal,
        }
        for i in range(NCORES)
    ]
    if TRACE:
        _enable_trace_hook()
    res = bass_utils.run_bass_kernel_spmd(
        nc,
        in_maps,
        core_ids=list(range(NCORES)),
        trace=TRACE,
        tmpdir=TRACE_DIR,
    )
    LAST_EXEC_NS = res.exec_time_ns
    LAST_RESULTS = res
    out = np.concatenate(
        [res.results[i]["out"].astype(np.float32) for i in range(NCORES)], axis=0
    )
    return out
